# revision 1
# baseline (speedup 1.0000x reference)
"""Trainium2 Bass kernel for nn_DistillSTU (LDS scan + spectral contraction).

Math: out[t,d] = sum_{delta>=0} k[delta,d] * u[t-delta,d],  u = x @ M_inputs,
      k[delta,d] = sum_j W[j,d]*Bm[j]*A[j]^delta (+ dvg[d] at delta=0),
      W = (C[:,:24]+C[:,24:]) @ M_filters, dvg = (Dv[:24]+Dv[24:]) @ M_filters.

Sharding: 768 channels split across 8 cores (96 each); embarrassingly parallel.

Per-core decomposition over T=2048 (chunks L=128, subs l=8):
  base   same-sub pairs (lag 0..7): exact short kernel; shift-FMA on
         ScalarE (lag 0) + GpSimd/VectorE (lags 1..7), d-partition layout.
  sub    same-chunk earlier-sub pairs: reduced-pole (r=8) states, batched
         across all chunks into 3 wide matmuls; one carry matmul per chunk.
  chunk  earlier-chunk pairs: exact 100 poles; chunk states via 3 wide
         matmuls + one tensor_tensor_scan; one carry matmul per chunk.
All state tensors use the (d,c)-interleaved free layout (col = d*NCH + c)
so per-chunk slices are stride-NCH column views.
"""
import sys
import numpy as np

sys.path.insert(0, "/opt/trn_rl_repo")

T = 2048
D = 768
NJ = 32           # reduced chunk-path state dim (fit is ~1e-11 exact)
L = 128           # chunk length
NCH = T // L      # 16 chunks
SUB = 8           # sub length
NS = L // SUB     # 16 subs per chunk
R = 8             # reduced poles for sub-carries; (s,p) = 15*8 = 120 <= 128
NCORE = 8
DP = D // NCORE   # 96 channels per core
FC = DP * NCH     # 1536 free cols of the (d,c) layout

_CACHE = {}

# column offsets inside the packed constant blocks (partition dim = 128)
_CONST_WIDTHS = [
    ("mi", 6 * DP), ("qt", NJ), ("pt4", 4 * L), ("rt", (NS - 1) * R),
    ("p2", (NS - 1) * SUB), ("ktab", SUB), ("ident", DP),
]
_CONST2_WIDTHS = [("gate", FC), ("wrep", FC), ("vrep", FC)]
CONST_OFF = {}
_off = 0
for _n, _w in _CONST_WIDTHS:
    CONST_OFF[_n] = _off
    _off += _w
CW = _off
CONST2_OFF = {}
_off = 0
for _n, _w in _CONST2_WIDTHS:
    CONST2_OFF[_n] = _off
    _off += _w
CW2 = _off


def _derive_tables(A, Bm, C, Dv, M_filters, M_inputs):
    """All host-side parameter preprocessing (small tensors only)."""
    f8 = np.float64
    A = A.astype(f8); Bm = Bm.astype(f8)
    C = C.astype(f8); Dv = Dv.astype(f8); Mf = M_filters.astype(f8)
    W = (C[:, :24] + C[:, 24:]) @ Mf                    # (100, 768)
    dvg = (Dv[:24] + Dv[24:]) @ Mf                      # (768,)
    V100 = W * Bm[:, None]                              # (100, 768)

    # exact short kernel (lags 0..7)
    pows = A[None, :] ** np.arange(SUB)[:, None]        # (8, 100)
    ktab8 = pows @ V100                                 # (8, 768)
    ktab8[0] += dvg

    # reduced-pole fit of k[delta,d] on delta in [1, L-1]; pole decay
    # rates refined by Nelder-Mead on the least-squares residual
    deltas = np.arange(1, L)
    kwin = (A[None, :] ** deltas[:, None]) @ V100       # (127, 768)

    def _fit(lam):
        mu = np.exp(-np.abs(lam))
        G = mu[None, :] ** deltas[:, None]
        Vr, *_ = np.linalg.lstsq(G, kwin, rcond=None)
        return mu, G, Vr, np.linalg.norm(G @ Vr - kwin)

    lam = np.geomspace(0.02, 1.5, R)
    mu, G, Vr, r0 = _fit(lam)
    try:
        from scipy.optimize import minimize
        res = minimize(lambda x: _fit(x)[3], lam, method="Nelder-Mead",
                       options={"maxiter": 3000, "fatol": 1e-12})
        mu2, G2, Vr2, r2 = _fit(res.x)
        if r2 < r0:
            mu, G, Vr = mu2, G2, Vr2
    except Exception:
        pass

    # chunk-level tables: 32 reduced poles fit on lags [1, 2047]
    d2 = np.arange(1, T)
    k2 = (A[None, :] ** d2[:, None]) @ V100             # (2047, 768)
    mu2 = np.exp(-np.geomspace(0.008, 3.0, NJ))
    G2 = mu2[None, :] ** d2[:, None]
    V2, *_ = np.linalg.lstsq(G2, k2, rcond=None)        # (32, 768)
    qt = mu2[None, :] ** (L - 1 - np.arange(L))[:, None]        # (128, 32)
    pt4 = np.zeros((4 * NJ, 4 * L))                     # block-diag carries
    ptb = np.ascontiguousarray((mu2[None, :] ** (np.arange(L) + 1)[:, None]).T)
    for c4 in range(4):
        pt4[c4 * NJ:(c4 + 1) * NJ, c4 * L:(c4 + 1) * L] = ptb
    gate = np.broadcast_to((mu2 ** L)[:, None], (NJ, FC)).copy()
    gate[:, 0::NCH] = 0.0                               # reset at c==0 per channel

    # sub-level tables (reduced poles); (s,p) order: s=1..15 outer, p inner
    rt = np.zeros((L, (NS - 1) * R))
    for s in range(1, NS):
        m = np.arange(SUB * s)
        rt[: SUB * s, (s - 1) * R:s * R] = mu[None, :] ** (SUB * s - 1 - m)[:, None]
    p2 = np.zeros(((NS - 1) * R, (NS - 1) * SUB))       # block-diag carries
    pr = mu[:, None] ** (np.arange(SUB) + 1)[None, :]   # (R, 8)
    for s in range(NS - 1):
        p2[s * R:(s + 1) * R, s * SUB:(s + 1) * SUB] = pr

    f4 = np.float32
    per_core = []
    for i in range(NCORE):
        sl = slice(i * DP, (i + 1) * DP)
        wrep = np.repeat(V2[:, sl][:, :, None], NCH, axis=2).reshape(NJ, FC)
        vrep = np.zeros(((NS - 1) * R, FC))
        vr_dc = np.repeat(Vr[:, sl][:, :, None], NCH, axis=2).reshape(R, FC)
        for s in range(NS - 1):
            vrep[s * R:(s + 1) * R] = vr_dc
        mi = np.ascontiguousarray(M_inputs.astype(f8)[:, sl]).astype(f4)
        ktabT = np.ascontiguousarray(ktab8[:, sl].T)    # (96, 8)
        cb = np.zeros((128, CW), dtype=f4)
        for name, arr in (
            ("mi", mi.reshape(6, 128, DP).transpose(1, 0, 2).reshape(128, 6 * DP)),
            ("qt", qt), ("pt4", pt4), ("rt", rt), ("p2", p2), ("ktab", ktabT),
            ("ident", np.eye(DP)),
        ):
            c0 = CONST_OFF[name]
            cb[:arr.shape[0], c0:c0 + arr.shape[1]] = arr
        cb2 = np.zeros((128, CW2), dtype=f4)
        for name, arr in (("gate", gate), ("wrep", wrep), ("vrep", vrep)):
            c0 = CONST2_OFF[name]
            cb2[:arr.shape[0], c0:c0 + arr.shape[1]] = arr
        per_core.append(dict(consts=cb, consts2=cb2))
    return per_core


def _build_nc():
    from concourse import bass, bacc, mybir, tile

    nc = bacc.Bacc()
    f4 = mybir.dt.float32
    xT = nc.declare_dram_parameter("xT", [D, T], f4, isOutput=False)
    cdram = nc.declare_dram_parameter("consts", [128, CW], f4, isOutput=False)
    cdram2 = nc.declare_dram_parameter("consts2", [128, CW2], f4, isOutput=False)
    out = nc.declare_dram_parameter("out", [DP, T], f4, isOutput=True)

    KT = D // L   # 6 k-tiles for the projection contraction
    NSPL = T // 512

    with tile.TileContext(nc) as tc:
        with (
            tc.tile_pool(name="consts", bufs=1) as consts,
            tc.tile_pool(name="xt", bufs=1) as xtp,
            tc.tile_pool(name="work", bufs=1) as work,
            tc.tile_pool(name="ps", bufs=2, space="PSUM") as psp,
            tc.tile_pool(name="big", bufs=3, space="PSUM") as bigp,
            tc.tile_pool(name="carry", bufs=3, space="PSUM") as cpool,
        ):
            call = consts.tile([128, CW], f4, tag="call")
            nc.sync.dma_start(call[:], cdram[:])
            call2 = consts.tile([128, CW2], f4, tag="call2")
            nc.sync.dma_start(call2[:], cdram2[:])

            def cs(name, rows, width, woff=0):
                c0 = CONST_OFF[name] + woff
                return call[0:rows, c0:c0 + width]

            def cs2(name, rows, width):
                c0 = CONST2_OFF[name]
                return call2[0:rows, c0:c0 + width]

            mi_sb = [cs("mi", 128, DP, k * DP) for k in range(KT)]
            qt_sb = cs("qt", L, NJ)
            pt4_sb = cs("pt4", 4 * NJ, 4 * L)
            rt_sb = cs("rt", L, (NS - 1) * R)
            p2_sb = cs("p2", (NS - 1) * R, (NS - 1) * SUB)
            ktab_sb = cs("ktab", DP, SUB)
            id_sb = cs("ident", DP, DP)
            gate_sb = cs2("gate", NJ, FC)
            wrep_sb = cs2("wrep", NJ, FC)
            vrep_sb = cs2("vrep", (NS - 1) * R, FC)

            xt_sb = []
            for k in range(KT):
                t = xtp.tile([L, T], f4, tag=f"xt{k}", name=f"xt_sb{k}")
                nc.sync.dma_start(t[:], xT[k * L:(k + 1) * L, :])
                xt_sb.append(t)

            u_dt = work.tile([DP, T], f4, tag="u_dt")
            u_tp = work.tile([L, FC], f4, tag="u_tp")
            s_all = work.tile([NJ, FC], f4, tag="s_all")
            e_all = work.tile([NJ, FC], f4, tag="e_all")
            f_all = work.tile([NJ, FC], f4, tag="f_all")
            f2_all = work.tile([(NS - 1) * R, FC], f4, tag="f2_all")
            f4sh = work.tile([4 * NJ, 4 * DP], f4, tag="f4sh")
            base_sb = work.tile([DP, T], f4, tag="base_sb")
            out_sb = work.tile([DP, T], f4, tag="out_sb")

            # ---- projection: u_dt[d, t] = sum_e mi[e, d] * xT[e, t]
            for n in range(NSPL):
                pu = psp.tile([DP, 512], f4, tag="ps")
                for k in range(KT):
                    nc.tensor.matmul(
                        pu[:], mi_sb[k], xt_sb[k][:, n * 512:(n + 1) * 512],
                        start=(k == 0), stop=(k == KT - 1))
                nc.scalar.copy(u_dt[:, n * 512:(n + 1) * 512], pu[:])

            # ---- base triangle (exact, lags 0..7) in d-partition layout.
            nc.scalar.activation(base_sb[:], u_dt[:],
                                 mybir.ActivationFunctionType.Copy,
                                 scale=ktab_sb[:, 0:1])
            for dlt in range(1, SUB):
                ov = base_sb[:].rearrange(
                    "d (sb l) -> d sb l", l=SUB)[:, :, dlt:SUB]
                uv = u_dt[:].rearrange(
                    "d (sb l) -> d sb l", l=SUB)[:, :, 0:SUB - dlt]
                nc.vector.scalar_tensor_tensor(
                    ov, uv, ktab_sb[:, dlt:dlt + 1], ov,
                    op0=mybir.AluOpType.mult, op1=mybir.AluOpType.add)

            # ---- u_tp: per-chunk transpose of u_dt, (d,c)-interleaved cols
            for c in range(NCH):
                ptp = psp.tile([L, DP], f4, tag="ps")
                nc.tensor.transpose(ptp[:], u_dt[:, c * L:(c + 1) * L], id_sb)
                nc.scalar.copy(u_tp[:, c::NCH], ptp[:])

            # ---- chunk states: 3 bank-sized matmuls + scan
            for n in range(3):
                sp = bigp.tile([NJ, 512], f4, tag="big", name=f"sp{n}")
                nc.tensor.matmul(sp[:], qt_sb,
                                 u_tp[:, n * 512:(n + 1) * 512],
                                 start=True, stop=True)
                nc.scalar.copy(s_all[:, n * 512:(n + 1) * 512], sp[:])
            nc.vector.tensor_tensor_scan(
                e_all[:], gate_sb, s_all[:], 0.0,
                op0=mybir.AluOpType.mult, op1=mybir.AluOpType.add)
            # f_all written (c,d)-blocked so chunk slices are contiguous
            nc.vector.tensor_tensor(
                f_all[:].rearrange("p (c d) -> p d c", d=DP),
                e_all[:].rearrange("p (d c) -> p d c", c=NCH),
                wrep_sb.rearrange("p (d c) -> p d c", c=NCH),
                op=mybir.AluOpType.mult)

            # ---- sub states: 3 bank-sized matmuls + fold fitted weights
            for n in range(3):
                ep = bigp.tile([(NS - 1) * R, 512], f4, tag="big", name=f"ep{n}")
                nc.tensor.matmul(ep[:], rt_sb,
                                 u_tp[:, n * 512:(n + 1) * 512],
                                 start=True, stop=True)
                nc.vector.tensor_tensor(
                    f2_all[:, n * 512:(n + 1) * 512], ep[:],
                    vrep_sb[:, n * 512:(n + 1) * 512],
                    op=mybir.AluOpType.mult)

            # ---- shuffle chunk states for 4x-batched carry matmuls:
            # f4sh[(c4, p), (g, d)] = f_all[p, (c=4g+c4-1, d)], zeros at c=0
            f4v = f4sh[:].rearrange("q (g d) -> q g d", d=DP)
            nc.vector.memset(f4sh[0:NJ, 0:DP], 0.0)
            fav = f_all[:].rearrange("p (c d) -> p c d", d=DP)
            nc.sync.dma_start(f4v[0:NJ, 1:4, :], fav[:, 3:12:4, :])
            for c4 in range(1, 4):
                nc.sync.dma_start(f4v[c4 * NJ:(c4 + 1) * NJ, :, :],
                                  fav[:, (c4 - 1)::4, :])

            # ---- per 4-chunk group: batched chunk carry + 4 sub carries,
            # then merge each chunk with the base and stream the output out
            for g in range(4):
                sacc = cpool.tile([DP, 4 * L], f4, tag="sacc", bufs=3)
                nc.tensor.matmul(sacc[:], f4sh[:, g * DP:(g + 1) * DP],
                                 pt4_sb, start=True, stop=False)
                for c4 in range(4):
                    c = 4 * g + c4
                    nc.tensor.matmul(
                        sacc[:, c4 * L + SUB:(c4 + 1) * L],
                        f2_all[:, c::NCH], p2_sb,
                        start=False, stop=(c4 == 3))
                for c4 in range(4):
                    c = 4 * g + c4
                    nc.vector.tensor_tensor(
                        out_sb[:, c * L:(c + 1) * L],
                        sacc[:, c4 * L:(c4 + 1) * L],
                        base_sb[:, c * L:(c + 1) * L], op=mybir.AluOpType.add)
                nc.sync.dma_start(out[:, g * 4 * L:(g + 1) * 4 * L],
                                  out_sb[:, g * 4 * L:(g + 1) * 4 * L])
    nc.compile()
    return nc


def _get_program():
    if "nc" not in _CACHE:
        _CACHE["nc"] = _build_nc()
    return _CACHE["nc"]


def kernel(x, input_pos, M_inputs, M_filters, A, Bm, C, Dv, _trace=False,
           _trace_kwargs=None):
    from concourse.bass_utils import run_bass_kernel_spmd

    x = np.asarray(x, dtype=np.float32)
    per_core = _derive_tables(
        np.asarray(A), np.asarray(Bm), np.asarray(C), np.asarray(Dv),
        np.asarray(M_filters), np.asarray(M_inputs))
    xTm = np.ascontiguousarray(x[0].T)                   # (768, 2048)

    nc = _get_program()
    in_maps = [dict(xT=xTm, **per_core[i]) for i in range(NCORE)]
    kw = dict(_trace_kwargs or {})
    res = run_bass_kernel_spmd(nc, in_maps, list(range(NCORE)),
                               trace=_trace, **kw)
    _CACHE["last_result"] = res
    full = np.concatenate([res.results[i]["out"] for i in range(NCORE)], axis=0)
    return np.ascontiguousarray(full.T)[None].astype(np.float32)


if __name__ == "__main__":
    rng = np.random.default_rng(0)
    ins = dict(
        x=rng.standard_normal((1, T, D), dtype=np.float32),
        input_pos=np.arange(T, dtype=np.int32),
        M_inputs=(rng.standard_normal((D, D)) * 0.02).astype(np.float32),
        M_filters=(rng.standard_normal((24, D)) * 0.02).astype(np.float32),
        A=rng.uniform(0, 0.99, 100).astype(np.float32),
        Bm=(rng.standard_normal(100) * 0.1).astype(np.float32),
        C=(rng.standard_normal((100, 48)) * 0.1).astype(np.float32),
        Dv=(rng.standard_normal(48) * 0.1).astype(np.float32),
    )
    got = kernel(**ins)
    print("kernel output", got.shape, got.dtype, float(np.abs(got).max()))



# revision 4
# speedup vs baseline: 1.4362x; 1.4362x over previous
"""Trainium2 Bass kernel for nn_DistillSTU (LDS scan + spectral contraction).

Math: out[t,d] = sum_{delta>=0} k[delta,d] * u[t-delta,d],  u = x @ M_inputs,
      k[delta,d] = sum_j W[j,d]*Bm[j]*A[j]^delta (+ dvg[d] at delta=0),
      W = (C[:,:24]+C[:,24:]) @ M_filters, dvg = (Dv[:24]+Dv[24:]) @ M_filters.

Sharding: 768 channels split across 8 cores (96 each); embarrassingly parallel.

Per-core decomposition over T=2048 (chunks L=128, subs l=8):
  base   same-sub pairs (lag 0..7): exact short kernel; lag0 on ScalarE from
         PSUM, lags 1-4 shift-FMA on VectorE, lags 5-7 on GpSimd.
  sub    same-chunk earlier-sub pairs: reduced-pole (r=8) states, batched
         across chunks into per-group matmuls; one carry matmul per chunk.
  chunk  earlier-chunk pairs: 32 fitted poles; states via per-group matmuls
         + one tensor_tensor_scan; batched carry matmuls per 4-chunk group.
All matmul operands are fp16 (1 cyc/row on the PE vs 4 for fp32); PSUM
accumulation stays fp32.  State tensors use the (d,c)-interleaved free
layout (col = d*NCH + c) so per-chunk slices are stride-NCH column views.
"""
import sys
import numpy as np

sys.path.insert(0, "/opt/trn_rl_repo")

T = 2048
D = 768
NJ = 32           # reduced chunk-path state dim (fit is ~1e-11 exact)
L = 128           # chunk length
NCH = T // L      # 16 chunks
SUB = 8           # sub length
NS = L // SUB     # 16 subs per chunk
R = 8             # reduced poles for sub-carries; (s,p) = 15*8 = 120 <= 128
NCORE = 8
DP = D // NCORE   # 96 channels per core
FC = DP * NCH     # 1536 free cols of the (d,c) layout
NSPL = T // 512   # 4 column splits of 512

_CACHE = {}

# column offsets inside the packed fp16 constant block (partition dim = 128)
_CONST_WIDTHS = [
    ("mi", 6 * DP), ("qt", NJ), ("pt4", 4 * L), ("rt", (NS - 1) * R),
    ("p2", (NS - 1) * SUB), ("ident", DP),
]
CONST_OFF = {}
_off = 0
for _n, _w in _CONST_WIDTHS:
    CONST_OFF[_n] = _off
    _off += _w
CW = _off


def _derive_tables(A, Bm, C, Dv, M_filters, M_inputs):
    """All host-side parameter preprocessing (small tensors only)."""
    f8 = np.float64
    A = A.astype(f8); Bm = Bm.astype(f8)
    C = C.astype(f8); Dv = Dv.astype(f8); Mf = M_filters.astype(f8)
    W = (C[:, :24] + C[:, 24:]) @ Mf                    # (100, 768)
    dvg = (Dv[:24] + Dv[24:]) @ Mf                      # (768,)
    V100 = W * Bm[:, None]                              # (100, 768)

    # exact short kernel (lags 0..7)
    pows = A[None, :] ** np.arange(SUB)[:, None]        # (8, 100)
    ktab8 = pows @ V100                                 # (8, 768)
    ktab8[0] += dvg

    # reduced-pole fit of k[delta,d] on delta in [1, L-1]; pole decay
    # rates refined by Nelder-Mead on the least-squares residual
    deltas = np.arange(1, L)
    kwin = (A[None, :] ** deltas[:, None]) @ V100       # (127, 768)

    def _fit(lam):
        mu = np.exp(-np.abs(lam))
        G = mu[None, :] ** deltas[:, None]
        Vr, *_ = np.linalg.lstsq(G, kwin, rcond=None)
        return mu, G, Vr, np.linalg.norm(G @ Vr - kwin)

    lam = np.geomspace(0.02, 1.5, R)
    mu, G, Vr, r0 = _fit(lam)
    try:
        from scipy.optimize import minimize
        res = minimize(lambda x: _fit(x)[3], lam, method="Nelder-Mead",
                       options={"maxiter": 3000, "fatol": 1e-12})
        mu2, G2, Vr2, r2 = _fit(res.x)
        if r2 < r0:
            mu, G, Vr = mu2, G2, Vr2
    except Exception:
        pass

    # chunk-level tables: 32 reduced poles fit on lags [1, 2047]
    d2 = np.arange(1, T)
    k2 = (A[None, :] ** d2[:, None]) @ V100             # (2047, 768)
    mu2 = np.exp(-np.geomspace(0.008, 3.0, NJ))
    G2 = mu2[None, :] ** d2[:, None]
    V2, *_ = np.linalg.lstsq(G2, k2, rcond=None)        # (32, 768)
    qt = mu2[None, :] ** (L - 1 - np.arange(L))[:, None]        # (128, 32)
    pt4 = np.zeros((4 * NJ, 4 * L))                     # block-diag carries
    ptb = np.ascontiguousarray((mu2[None, :] ** (np.arange(L) + 1)[:, None]).T)
    for c4 in range(4):
        pt4[c4 * NJ:(c4 + 1) * NJ, c4 * L:(c4 + 1) * L] = ptb
    gate = np.broadcast_to((mu2 ** L)[:, None], (NJ, FC)).copy()
    gate[:, 0::NCH] = 0.0                               # reset at c==0 per channel

    # sub-level tables (reduced poles); (s,p) order: s=1..15 outer, p inner
    rt = np.zeros((L, (NS - 1) * R))
    for s in range(1, NS):
        m = np.arange(SUB * s)
        rt[: SUB * s, (s - 1) * R:s * R] = mu[None, :] ** (SUB * s - 1 - m)[:, None]
    p2 = np.zeros(((NS - 1) * R, (NS - 1) * SUB))       # block-diag carries
    pr = mu[:, None] ** (np.arange(SUB) + 1)[None, :]   # (R, 8)
    for s in range(NS - 1):
        p2[s * R:(s + 1) * R, s * SUB:(s + 1) * SUB] = pr

    f4 = np.float32
    f2 = np.float16
    per_core = []
    for i in range(NCORE):
        sl = slice(i * DP, (i + 1) * DP)
        wrep = np.repeat(V2[:, sl][:, :, None], NCH, axis=2).reshape(NJ, FC)
        vrep = np.zeros(((NS - 1) * R, FC))
        vr_dc = np.repeat(Vr[:, sl][:, :, None], NCH, axis=2).reshape(R, FC)
        for s in range(NS - 1):
            vrep[s * R:(s + 1) * R] = vr_dc
        mi = np.ascontiguousarray(M_inputs.astype(f8)[:, sl])
        ktabT = np.ascontiguousarray(ktab8[:, sl].T)    # (96, 8)
        cb = np.zeros((128, CW), dtype=f2)
        for name, arr in (
            ("mi", mi.reshape(6, 128, DP).transpose(1, 0, 2).reshape(128, 6 * DP)),
            ("qt", qt), ("pt4", pt4), ("rt", rt), ("p2", p2),
            ("ident", np.eye(DP)),
        ):
            c0 = CONST_OFF[name]
            cb[:arr.shape[0], c0:c0 + arr.shape[1]] = arr.astype(f2)
        per_core.append(dict(
            consts=cb,
            ktab=np.ascontiguousarray(ktabT.astype(f4)),
            gate=gate.astype(f2),
            wrep=wrep.astype(f2),
            vrep=vrep.astype(f2),
        ))
    return per_core


def _build_nc():
    from concourse import bass, bacc, mybir, tile

    nc = bacc.Bacc()
    f4 = mybir.dt.float32
    f2 = mybir.dt.float16
    xT = nc.declare_dram_parameter("xT", [D, T], f2, isOutput=False)
    cdram = nc.declare_dram_parameter("consts", [128, CW], f2, isOutput=False)
    kdram = nc.declare_dram_parameter("ktab", [DP, SUB], f4, isOutput=False)
    gdram = nc.declare_dram_parameter("gate", [NJ, FC], f2, isOutput=False)
    wdram = nc.declare_dram_parameter("wrep", [NJ, FC], f2, isOutput=False)
    vdram = nc.declare_dram_parameter("vrep", [(NS - 1) * R, FC], f2,
                                      isOutput=False)
    out = nc.declare_dram_parameter("out", [DP, T], f4, isOutput=True)

    KT = D // L   # 6 k-tiles for the projection contraction
    Copy = mybir.ActivationFunctionType.Copy
    MUL = mybir.AluOpType.mult
    ADD = mybir.AluOpType.add

    with tile.TileContext(nc) as tc:
        with (
            tc.tile_pool(name="consts", bufs=1) as consts,
            tc.tile_pool(name="xt", bufs=1) as xtp,
            tc.tile_pool(name="work", bufs=1) as work,
            tc.tile_pool(name="pj", bufs=2, space="PSUM") as pjp,
            tc.tile_pool(name="tp", bufs=2, space="PSUM") as tpp,
            tc.tile_pool(name="st", bufs=2, space="PSUM") as stp,
            tc.tile_pool(name="cr", bufs=2, space="PSUM") as crp,
        ):
            call = consts.tile([128, CW], f2, tag="call")
            nc.sync.dma_start(call[:], cdram[:])
            ktab_sb = consts.tile([DP, SUB], f4, tag="ktab")
            nc.sync.dma_start(ktab_sb[:], kdram[:])
            gate_sb = consts.tile([NJ, FC], f2, tag="gate")
            nc.sync.dma_start(gate_sb[:], gdram[:])
            wrep_sb = consts.tile([NJ, FC], f2, tag="wrep")
            nc.sync.dma_start(wrep_sb[:], wdram[:])
            vrep_sb = consts.tile([(NS - 1) * R, FC], f2, tag="vrep")
            nc.sync.dma_start(vrep_sb[:], vdram[:])

            def cs(name, rows, width, woff=0):
                c0 = CONST_OFF[name] + woff
                return call[0:rows, c0:c0 + width]

            mi_sb = [cs("mi", 128, DP, k * DP) for k in range(KT)]
            qt_sb = cs("qt", L, NJ)
            pt4_sb = cs("pt4", 4 * NJ, 4 * L)
            rt_sb = cs("rt", L, (NS - 1) * R)
            p2_sb = cs("p2", (NS - 1) * R, (NS - 1) * SUB)
            id_sb = cs("ident", DP, DP)

            # xT chunk DMAs, n-major so projection group 0 starts early
            xt_sb = []
            for k in range(KT):
                t = xtp.tile([L, T], f2, tag=f"xt{k}", name=f"xt_sb{k}")
                xt_sb.append(t)
            for n in range(NSPL):
                for k in range(KT):
                    nc.sync.dma_start(
                        xt_sb[k][:, n * 512:(n + 1) * 512],
                        xT[k * L:(k + 1) * L, n * 512:(n + 1) * 512])

            u_bf = work.tile([DP, T], f2, tag="u_bf")
            u_tp = work.tile([L, FC], f2, tag="u_tp")
            s_all = work.tile([NJ, FC], f2, tag="s_all")
            e_all = work.tile([NJ, FC], f2, tag="e_all")
            f_all = work.tile([NJ, FC], f2, tag="f_all")
            f2_all = work.tile([(NS - 1) * R, FC], f2, tag="f2_all")
            f4sh = work.tile([4 * NJ, 4 * DP], f2, tag="f4sh")
            base_sb = work.tile([DP, T], f2, tag="base_sb")
            out_sb = work.tile([DP, T], f4, tag="out_sb")

            # ---- projection u[d, t] = sum_e mi[e, d] * xT[e, t], n-outer
            # so each 512-col split flows to base/transpose work while the
            # next split's matmuls run.
            for n in range(NSPL):
                pu = pjp.tile([DP, 512], f4, tag="pj", name=f"pu{n}")
                for k in range(KT):
                    nc.tensor.matmul(
                        pu[:], mi_sb[k], xt_sb[k][:, n * 512:(n + 1) * 512],
                        start=(k == 0), stop=(k == KT - 1))
                nc.scalar.copy(u_bf[:, n * 512:(n + 1) * 512], pu[:])
                # base lag 0 straight from PSUM with per-channel scale
                nc.scalar.activation(base_sb[:, n * 512:(n + 1) * 512], pu[:],
                                     Copy, scale=ktab_sb[:, 0:1])

                # base triangle lags 1..7 (shift-FMA inside each sub of 8):
                # lags 1-4 on VectorE, lags 5-7 on GpSimd
                bv = base_sb[:, n * 512:(n + 1) * 512].rearrange(
                    "d (sb l) -> d sb l", l=SUB)
                uv = u_bf[:, n * 512:(n + 1) * 512].rearrange(
                    "d (sb l) -> d sb l", l=SUB)
                for dlt in range(1, SUB):
                    eng = nc.vector
                    eng.scalar_tensor_tensor(
                        bv[:, :, dlt:SUB], uv[:, :, 0:SUB - dlt],
                        ktab_sb[:, dlt:dlt + 1], bv[:, :, dlt:SUB],
                        op0=MUL, op1=ADD)

                # per-chunk transposes of this split (4 chunks -> 1 bank)
                g = n
                ptp = tpp.tile([L, 4 * DP], f2, tag="tp", name=f"ptp{g}")
                for c4 in range(4):
                    c = 4 * g + c4
                    nc.tensor.transpose(
                        ptp[:, c4 * DP:(c4 + 1) * DP],
                        u_bf[:, c * L:(c + 1) * L], id_sb)
                # one strided copy into the (d,c)-interleaved layout
                nc.vector.tensor_copy(
                    u_tp[:].rearrange("p (d c) -> p d c", c=NCH)[
                        :, :, 4 * g:4 * g + 4],
                    ptp[:].rearrange("p (c d) -> p d c", d=DP))

                # state matmuls for this chunk group (cols c in group, all d)
                mv = u_tp[:].rearrange("p (d c) -> p d c", c=NCH)[
                    :, :, 4 * g:4 * g + 4]
                sp = stp.tile([NJ, 4 * DP], f4, tag="st", name=f"sp{g}")
                nc.tensor.matmul(sp[:], qt_sb, mv, start=True, stop=True)
                nc.scalar.copy(
                    s_all[:].rearrange("p (d c) -> p d c", c=NCH)[
                        :, :, 4 * g:4 * g + 4],
                    sp[:].rearrange("p (d c) -> p d c", c=4))
                ep = stp.tile([(NS - 1) * R, 4 * DP], f4, tag="st",
                              name=f"ep{g}")
                nc.tensor.matmul(ep[:], rt_sb, mv, start=True, stop=True)
                nc.vector.tensor_tensor(
                    f2_all[:].rearrange("p (d c) -> p d c", c=NCH)[
                        :, :, 4 * g:4 * g + 4],
                    ep[:].rearrange("p (d c) -> p d c", c=4),
                    vrep_sb[:].rearrange("p (d c) -> p d c", c=NCH)[
                        :, :, 4 * g:4 * g + 4],
                    op=MUL)

            # ---- chunk-state scan over the (d,c) layout; gate zeros at
            # c==0 make the initial value irrelevant.
            nc.vector.tensor_tensor_scan(
                e_all[:], gate_sb[:], s_all[:], 0.0, op0=MUL, op1=ADD)
            # f_all written (c,d)-blocked so chunk slices are contiguous
            nc.gpsimd.tensor_tensor(
                f_all[:].rearrange("p (c d) -> p d c", d=DP),
                e_all[:].rearrange("p (d c) -> p d c", c=NCH),
                wrep_sb[:].rearrange("p (d c) -> p d c", c=NCH),
                op=MUL)

            # ---- shuffle chunk states for 4x-batched carry matmuls:
            # f4sh[(c4, p), (g, d)] = f_all[p, (c=4g+c4-1, d)], zeros at c=0
            f4v = f4sh[:].rearrange("q (g d) -> q g d", d=DP)
            nc.vector.memset(f4sh[0:NJ, 0:DP], 0.0)
            fav = f_all[:].rearrange("p (c d) -> p c d", d=DP)
            nc.sync.dma_start(f4v[0:NJ, 1:4, :], fav[:, 3:12:4, :])
            for c4 in range(1, 4):
                nc.sync.dma_start(f4v[c4 * NJ:(c4 + 1) * NJ, :, :],
                                  fav[:, (c4 - 1)::4, :])

            # ---- per 4-chunk group: batched chunk carry + 4 sub carries,
            # then merge each chunk with the base and stream the output out
            for g in range(4):
                sacc = crp.tile([DP, 4 * L], f4, tag="sacc", name=f"sacc{g}")
                nc.tensor.matmul(sacc[:], f4sh[:, g * DP:(g + 1) * DP],
                                 pt4_sb, start=True, stop=False)
                for c4 in range(4):
                    c = 4 * g + c4
                    nc.tensor.matmul(
                        sacc[:, c4 * L + SUB:(c4 + 1) * L],
                        f2_all[:, c::NCH], p2_sb,
                        start=False, stop=(c4 == 3))
                nc.vector.tensor_tensor(
                    out_sb[:, g * 512:(g + 1) * 512], sacc[:],
                    base_sb[:, g * 512:(g + 1) * 512], op=ADD)
                nc.sync.dma_start(out[:, g * 512:(g + 1) * 512],
                                  out_sb[:, g * 512:(g + 1) * 512])
    nc.compile()
    return nc


def _get_program():
    if "nc" not in _CACHE:
        _CACHE["nc"] = _build_nc()
    return _CACHE["nc"]


def kernel(x, input_pos, M_inputs, M_filters, A, Bm, C, Dv, _trace=False,
           _trace_kwargs=None):
    from concourse.bass_utils import run_bass_kernel_spmd

    x = np.asarray(x, dtype=np.float32)
    per_core = _derive_tables(
        np.asarray(A), np.asarray(Bm), np.asarray(C), np.asarray(Dv),
        np.asarray(M_filters), np.asarray(M_inputs))
    xTm = np.ascontiguousarray(x[0].T).astype(np.float16)   # (768, 2048)

    nc = _get_program()
    in_maps = [dict(xT=xTm, **per_core[i]) for i in range(NCORE)]
    kw = dict(_trace_kwargs or {})
    res = run_bass_kernel_spmd(nc, in_maps, list(range(NCORE)),
                               trace=_trace, **kw)
    _CACHE["last_result"] = res
    full = np.concatenate([res.results[i]["out"] for i in range(NCORE)], axis=0)
    return np.ascontiguousarray(full.T)[None].astype(np.float32)


if __name__ == "__main__":
    rng = np.random.default_rng(0)
    ins = dict(
        x=rng.standard_normal((1, T, D), dtype=np.float32),
        input_pos=np.arange(T, dtype=np.int32),
        M_inputs=(rng.standard_normal((D, D)) * 0.02).astype(np.float32),
        M_filters=(rng.standard_normal((24, D)) * 0.02).astype(np.float32),
        A=rng.uniform(0, 0.99, 100).astype(np.float32),
        Bm=(rng.standard_normal(100) * 0.1).astype(np.float32),
        C=(rng.standard_normal((100, 48)) * 0.1).astype(np.float32),
        Dv=(rng.standard_normal(48) * 0.1).astype(np.float32),
    )
    got = kernel(**ins)
    print("kernel output", got.shape, got.dtype, float(np.abs(got).max()))


# revision 9
# speedup vs baseline: 1.5097x; 1.0512x over previous
"""Trainium2 Bass kernel for nn_DistillSTU (LDS scan + spectral contraction).

Math: out[t,d] = sum_{delta>=0} k[delta,d] * u[t-delta,d],  u = x @ M_inputs,
      k[delta,d] = sum_j W[j,d]*Bm[j]*A[j]^delta (+ dvg[d] at delta=0),
      W = (C[:,:24]+C[:,24:]) @ M_filters, dvg = (Dv[:24]+Dv[24:]) @ M_filters.

Sharding: 768 channels split across 8 cores (96 each); embarrassingly parallel.

Layout: the whole kernel runs in an "l-major" time permutation.  Within each
512-col split, host-permuted xT columns are ordered (l, sub) instead of
(sub, l), so the base-triangle shift-FMAs on VectorE see contiguous runs of
subs instead of 1-7-element strided windows.  Chunk-local time tl = sb*8+l
appears on-device as tlp = l*16+sb; qt/rt/ptb/p2 tables are row/col-permuted
on the host to match, and the host un-permutes the output columns.

Per-core decomposition over T=2048 (chunks L=128, subs of 8):
  base   same-sub pairs (lag 0..7): exact, VectorE shift-FMA, contiguous.
  sub    same-chunk earlier-sub pairs: reduced-pole (r=8) states via one
         matmul per 4-chunk group; one sub-carry matmul per chunk.
  chunk  earlier-chunk pairs: 32 fitted poles; states via one matmul per
         group, chunk recurrence as 16 chained per-chunk FMAs (contiguous
         (c,d)-blocked layout), per-chunk carry matmuls.
All matmul operands fp16 (1 cyc/row); PSUM accumulation fp32.  DMAs are
split across the two HWDGE rings (sync + scalar) to halve issue cost.
"""
import sys
import numpy as np

sys.path.insert(0, "/opt/trn_rl_repo")

T = 2048
D = 768
NJ = 32           # reduced chunk-path state dim
L = 128           # chunk length
NCH = T // L      # 16 chunks
SUB = 8           # sub length
NS = L // SUB     # 16 subs per chunk
R = 8             # reduced poles for sub-carries; (s,p) = 15*8 = 120 <= 128
NCORE = 8
DP = D // NCORE   # 96 channels per core
FC = DP * NCH     # 1536 cols of the (c,d)-blocked layout
NSPL = T // 512   # 4 column splits / chunk groups

_CACHE = {}

# column offsets inside the packed fp16 constant block (partition dim = 128)
_CONST_WIDTHS = [
    ("mi", 6 * DP), ("qt", NJ), ("ptb", L), ("rt", (NS - 1) * R),
    ("p2", 8 * (NS - 1)), ("ident", DP), ("vrep", FC),
]
CONST_OFF = {}
_off = 0
for _n, _w in _CONST_WIDTHS:
    CONST_OFF[_n] = _off
    _off += _w
CW = _off


def _derive_tables(A, Bm, C, Dv, M_filters, M_inputs):
    """All host-side parameter preprocessing (small tensors only)."""
    f8 = np.float64
    A = A.astype(f8); Bm = Bm.astype(f8)
    C = C.astype(f8); Dv = Dv.astype(f8); Mf = M_filters.astype(f8)
    W = (C[:, :24] + C[:, 24:]) @ Mf                    # (100, 768)
    dvg = (Dv[:24] + Dv[24:]) @ Mf                      # (768,)
    V100 = W * Bm[:, None]                              # (100, 768)

    # exact short kernel (lags 0..7)
    pows = A[None, :] ** np.arange(SUB)[:, None]        # (8, 100)
    ktab8 = pows @ V100                                 # (8, 768)
    ktab8[0] += dvg

    # reduced-pole fit of k[delta,d] on delta in [1, L-1]
    deltas = np.arange(1, L)
    kwin = (A[None, :] ** deltas[:, None]) @ V100       # (127, 768)

    def _fit(lam):
        mu = np.exp(-np.abs(lam))
        G = mu[None, :] ** deltas[:, None]
        Vr, *_ = np.linalg.lstsq(G, kwin, rcond=None)
        return mu, G, Vr, np.linalg.norm(G @ Vr - kwin)

    lam = np.geomspace(0.02, 1.5, R)
    mu, G, Vr, r0 = _fit(lam)
    try:
        from scipy.optimize import minimize
        res = minimize(lambda v: _fit(v)[3], lam, method="Nelder-Mead",
                       options={"maxiter": 3000, "fatol": 1e-12})
        mu2_, G2_, Vr2_, r2 = _fit(res.x)
        if r2 < r0:
            mu, G, Vr = mu2_, G2_, Vr2_
    except Exception:
        pass

    # chunk-level tables: 32 reduced poles fit on lags [1, 2047]
    d2 = np.arange(1, T)
    k2 = (A[None, :] ** d2[:, None]) @ V100             # (2047, 768)
    mu2 = np.exp(-np.geomspace(0.008, 3.0, NJ))
    G2 = mu2[None, :] ** d2[:, None]
    V2, *_ = np.linalg.lstsq(G2, k2, rcond=None)        # (32, 768)

    # l-major permutation: device-local index tlp = l*16 + sb, tl = sb*8 + l
    tl_of = (np.arange(L) % 16) * 8 + np.arange(L) // 16

    qt_perm = mu2[None, :] ** (L - 1 - tl_of)[:, None]          # (128, 32)
    ptb_perm = mu2[:, None] ** (tl_of + 1)[None, :]             # (32, 128)
    rt_perm = np.zeros((L, (NS - 1) * R))
    for i, tl in enumerate(tl_of):
        for s in range(1, NS):
            if tl < SUB * s:
                rt_perm[i, (s - 1) * R:s * R] = mu ** (SUB * s - 1 - tl)
    p2_perm = np.zeros(((NS - 1) * R, 8 * (NS - 1)))    # cols (l, sb-1)
    for l in range(8):
        for sb in range(1, NS):
            p2_perm[(sb - 1) * R:sb * R, l * (NS - 1) + (sb - 1)] = \
                mu ** (l + 1)
    gcol = (mu2 ** L)[:, None]                          # (32, 1)

    f4 = np.float32
    f2 = np.float16
    per_core = []
    for i in range(NCORE):
        sl = slice(i * DP, (i + 1) * DP)
        mi = np.ascontiguousarray(M_inputs.astype(f8)[:, sl])
        ktabT = np.ascontiguousarray(ktab8[:, sl].T)    # (96, 8)
        vrep_cd = np.concatenate(
            [np.tile(Vr[:, sl], (1, NCH))] * (NS - 1), axis=0)  # (120, 1536)
        cb = np.zeros((128, CW), dtype=f2)
        for name, arr in (
            ("mi", mi.reshape(6, 128, DP).transpose(1, 0, 2).reshape(128, 6 * DP)),
            ("qt", qt_perm), ("ptb", ptb_perm), ("rt", rt_perm),
            ("p2", p2_perm), ("ident", np.eye(DP)), ("vrep", vrep_cd),
        ):
            c0 = CONST_OFF[name]
            cb[:arr.shape[0], c0:c0 + arr.shape[1]] = arr.astype(f2)
        cf = np.zeros((DP, SUB + 1), dtype=f4)
        cf[:, :SUB] = ktabT
        cf[:NJ, SUB:SUB + 1] = gcol
        per_core.append(dict(
            consts=cb, cf=cf,
            wrep=np.tile(V2[:, sl], (1, NCH)).astype(f2),   # (32, 1536)
        ))
    return per_core


def _build_nc():
    from concourse import bass, bacc, mybir, tile

    nc = bacc.Bacc()
    f4 = mybir.dt.float32
    f2 = mybir.dt.float16
    xT = nc.declare_dram_parameter("xT", [D, T], f2, isOutput=False)
    cdram = nc.declare_dram_parameter("consts", [128, CW], f2, isOutput=False)
    fdram = nc.declare_dram_parameter("cf", [DP, SUB + 1], f4, isOutput=False)
    wdram = nc.declare_dram_parameter("wrep", [NJ, FC], f2, isOutput=False)
    out = nc.declare_dram_parameter("out", [DP, T], f4, isOutput=True)

    KT = D // L   # 6 k-tiles for the projection contraction
    MUL = mybir.AluOpType.mult
    ADD = mybir.AluOpType.add

    with tile.TileContext(nc) as tc:
        with (
            tc.tile_pool(name="consts", bufs=1) as consts,
            tc.tile_pool(name="xt", bufs=1) as xtp,
            tc.tile_pool(name="work", bufs=1) as work,
            tc.tile_pool(name="pj", bufs=2, space="PSUM") as pjp,
            tc.tile_pool(name="tp", bufs=2, space="PSUM") as tpp,
            tc.tile_pool(name="st", bufs=2, space="PSUM") as stp,
            tc.tile_pool(name="cr", bufs=2, space="PSUM") as crp,
        ):
            # consts on the scalar (ACT) HWDGE ring, xT on the sync ring
            cf_sb = consts.tile([DP, SUB + 1], f4, tag="cf")
            nc.scalar.dma_start(cf_sb[:], fdram[:])
            call = consts.tile([128, CW], f2, tag="call")
            nc.scalar.dma_start(call[:], cdram[:])
            wrep_sb = consts.tile([NJ, FC], f2, tag="wrep")
            nc.scalar.dma_start(wrep_sb[:], wdram[:])

            def cs(name, rows, width, woff=0):
                c0 = CONST_OFF[name] + woff
                return call[0:rows, c0:c0 + width]

            mi_sb = [cs("mi", 128, DP, k * DP) for k in range(KT)]
            qt_sb = cs("qt", L, NJ)
            ptb_sb = cs("ptb", NJ, L)
            rt_sb = cs("rt", L, (NS - 1) * R)
            p2_sb = cs("p2", (NS - 1) * R, 8 * (NS - 1))
            id_sb = cs("ident", DP, DP)
            vrep_sb = cs("vrep", (NS - 1) * R, FC)

            xt_sb = []
            for k in range(KT):
                t = xtp.tile([L, T], f2, tag=f"xt{k}", name=f"xt_sb{k}")
                xt_sb.append(t)
            for h in range(2):
                for k in range(KT):
                    nc.sync.dma_start(
                        xt_sb[k][:, h * 1024:(h + 1) * 1024],
                        xT[k * L:(k + 1) * L, h * 1024:(h + 1) * 1024])

            u_lsb = work.tile([DP, T], f2, tag="u_lsb")
            u_ch = work.tile([DP, T], f2, tag="u_ch")
            base_sb = work.tile([DP, T], f2, tag="base_sb")
            u_tp = work.tile([L, FC], f2, tag="u_tp")
            e_all = work.tile([NJ, FC], f2, tag="e_all")
            f_all = work.tile([NJ, FC], f2, tag="f_all")
            f2_all = work.tile([(NS - 1) * R, FC], f2, tag="f2_all")
            out_sb = work.tile([DP, T], f4, tag="out_sb")

            # l-major 3D views of the 512-col splits: dims (l 8, sub 64)
            def lview(tile_, n):
                return tile_[:].rearrange("d (l s) -> d l s", s=256)[
                    :, :, 64 * n:64 * (n + 1)]

            pu_t = [None] * NSPL
            ptp_t = [None] * NSPL
            sp_t = [None] * NSPL
            ep_t = [None] * NSPL

            for s in range(7):
                # ---- carries for group m=s-3: per-chunk carry matmuls,
                # merge with base, stream out
                if 3 <= s:
                    m = s - 3
                    sacc = crp.tile([DP, 512], f4, tag="sacc", name=f"sacc{m}")
                    if m == 0:
                        # cols (l,sb=0) of chunk 0 are never written by a
                        # matmul; zero them before the merge
                        nc.vector.memset(
                            sacc[:, 0:L].rearrange(
                                "d (l sb) -> d l sb", sb=16)[:, :, 0:1], 0.0)
                    first = True
                    for c4 in range(4):
                        c = 4 * m + c4
                        if c > 0:
                            nc.tensor.matmul(
                                sacc[:, c4 * L:(c4 + 1) * L],
                                f_all[:, (c - 1) * DP:c * DP], ptb_sb,
                                start=first, stop=False)
                            first = False
                        nc.tensor.matmul(
                            sacc[:, c4 * L:(c4 + 1) * L].rearrange(
                                "d (l sb) -> d l sb", sb=16)[:, :, 1:16],
                            f2_all[:, c * DP:(c + 1) * DP], p2_sb,
                            start=first, stop=(c4 == 3))
                        first = False
                    bv = base_sb[:].rearrange(
                        "d (l c sb) -> d c l sb", c=NCH, sb=16)[
                        :, 4 * m:4 * m + 4]
                    nc.vector.tensor_tensor(
                        out_sb[:, m * 512:(m + 1) * 512].rearrange(
                            "d (c l sb) -> d c l sb", l=8, sb=16),
                        sacc[:].rearrange("d (c l sb) -> d c l sb", l=8, sb=16),
                        bv, op=ADD)
                    nc.scalar.dma_start(out[:, m * 512:(m + 1) * 512],
                                        out_sb[:, m * 512:(m + 1) * 512])

                # ---- chunk-group states for m=s-2 + e-chain + f tensors
                if 2 <= s <= 5:
                    m = s - 2
                    sp = stp.tile([NJ, 4 * DP], f4, tag="st", name=f"sp{m}")
                    nc.tensor.matmul(sp[:], qt_sb,
                                     u_tp[:, m * 384:(m + 1) * 384],
                                     start=True, stop=True)
                    ep = stp.tile([(NS - 1) * R, 4 * DP], f4, tag="st",
                                  name=f"ep{m}")
                    nc.tensor.matmul(ep[:], rt_sb,
                                     u_tp[:, m * 384:(m + 1) * 384],
                                     start=True, stop=True)
                    sp_t[m], ep_t[m] = sp, ep
                    for c4 in range(4):
                        c = 4 * m + c4
                        if c == 0:
                            nc.vector.tensor_copy(e_all[:, 0:DP],
                                                  sp[:, 0:DP])
                        else:
                            nc.vector.scalar_tensor_tensor(
                                e_all[:, c * DP:(c + 1) * DP],
                                e_all[:, (c - 1) * DP:c * DP],
                                cf_sb[0:NJ, SUB:SUB + 1],
                                sp[:, c4 * DP:(c4 + 1) * DP],
                                op0=MUL, op1=ADD)
                    nc.vector.tensor_tensor(
                        f2_all[:, m * 384:(m + 1) * 384], ep[:],
                        vrep_sb[:, m * 384:(m + 1) * 384], op=MUL)
                    nc.vector.tensor_tensor(
                        f_all[:, m * 384:(m + 1) * 384],
                        e_all[:, m * 384:(m + 1) * 384],
                        wrep_sb[:, m * 384:(m + 1) * 384], op=MUL)

                # ---- transposes for group m=s-1 into (c,d)-blocked u_tp
                if 1 <= s <= 4:
                    m = s - 1
                    ptp = tpp.tile([L, 4 * DP], f2, tag="tp", name=f"ptp{m}")
                    for c4 in range(4):
                        c = 4 * m + c4
                        nc.tensor.transpose(
                            ptp[:, c4 * DP:(c4 + 1) * DP],
                            u_ch[:, c * L:(c + 1) * L], id_sb)
                    ptp_t[m] = ptp
                    nc.vector.tensor_copy(u_tp[:, m * 384:(m + 1) * 384],
                                          ptp[:])

                # ---- projection split n=s (moving cols already l-major)
                if s < NSPL:
                    n = s
                    pu = pjp.tile([DP, 512], f4, tag="pj", name=f"pu{n}")
                    for k in range(KT):
                        nc.tensor.matmul(
                            pu[:], mi_sb[k],
                            xt_sb[k][:, n * 512:(n + 1) * 512],
                            start=(k == 0), stop=(k == KT - 1))
                    pu_t[n] = pu
                    puv = pu[:].rearrange("d (l s) -> d l s", s=64)
                    nc.scalar.copy(lview(u_lsb, n), puv)
                    # chunk-contiguous copy for the PE transposes:
                    # u_ch col = c*128 + l*16 + sb <- psum col l*64 + c4*16 + sb
                    nc.scalar.copy(
                        u_ch[:, n * 512:(n + 1) * 512].rearrange(
                            "d (c l sb) -> d c l sb", l=8, sb=16),
                        pu[:].rearrange("d (l c sb) -> d c l sb", c=4, sb=16))
                    # base: lag 0 then shift-FMA lags 1..7, all contiguous
                    bvn = lview(base_sb, n)
                    uvn = lview(u_lsb, n)
                    nc.vector.tensor_scalar(
                        bvn, uvn, cf_sb[0:DP, 0:1], None, op0=MUL)
                    for dlt in range(1, SUB):
                        nc.vector.scalar_tensor_tensor(
                            bvn[:, dlt:SUB, :], uvn[:, 0:SUB - dlt, :],
                            cf_sb[0:DP, dlt:dlt + 1], bvn[:, dlt:SUB, :],
                            op0=MUL, op1=ADD)
    nc.compile()
    return nc


def _get_program():
    if "nc" not in _CACHE:
        _CACHE["nc"] = _build_nc()
    return _CACHE["nc"]


def kernel(x, input_pos, M_inputs, M_filters, A, Bm, C, Dv, _trace=False,
           _trace_kwargs=None):
    from concourse.bass_utils import run_bass_kernel_spmd

    x = np.asarray(x, dtype=np.float32)
    per_core = _derive_tables(
        np.asarray(A), np.asarray(Bm), np.asarray(C), np.asarray(Dv),
        np.asarray(M_filters), np.asarray(M_inputs))
    # host: transpose + per-512-split l-major permutation of the columns
    xTm = np.ascontiguousarray(x[0].T)                   # (768, 2048)
    xlm = np.ascontiguousarray(
        xTm.reshape(D, NSPL, 64, SUB).transpose(0, 1, 3, 2).reshape(D, T)
    ).astype(np.float16)

    nc = _get_program()
    in_maps = [dict(xT=xlm, **per_core[i]) for i in range(NCORE)]
    kw = dict(_trace_kwargs or {})
    res = run_bass_kernel_spmd(nc, in_maps, list(range(NCORE)),
                               trace=_trace, **kw)
    _CACHE["last_result"] = res
    full = np.concatenate([res.results[i]["out"] for i in range(NCORE)], axis=0)
    # un-permute: device cols (c, l, sb) -> t = c*128 + sb*8 + l
    full = full.reshape(D, NCH, 8, 16).transpose(0, 1, 3, 2).reshape(D, T)
    return np.ascontiguousarray(full.T)[None].astype(np.float32)


if __name__ == "__main__":
    rng = np.random.default_rng(0)
    ins = dict(
        x=rng.standard_normal((1, T, D), dtype=np.float32),
        input_pos=np.arange(T, dtype=np.int32),
        M_inputs=(rng.standard_normal((D, D)) * 0.02).astype(np.float32),
        M_filters=(rng.standard_normal((24, D)) * 0.02).astype(np.float32),
        A=rng.uniform(0, 0.99, 100).astype(np.float32),
        Bm=(rng.standard_normal(100) * 0.1).astype(np.float32),
        C=(rng.standard_normal((100, 48)) * 0.1).astype(np.float32),
        Dv=(rng.standard_normal(48) * 0.1).astype(np.float32),
    )
    got = kernel(**ins)
    print("kernel output", got.shape, got.dtype, float(np.abs(got).max()))
